# revision 19
# baseline (speedup 1.0000x reference)
"""Trainium2 Bass kernel for nn_CausalFeatureTransformer (v5).

Only the last sequence position (label token) of the reference output is needed,
so the per-sample transformer collapses (see derivation in v1):

  X_norm[n,f,:] = s[n,f]*u[f,:]*g1 + beta1  (f<256),  X_norm[n,256,:] = xlast
  u = feat_emb - rowmean(feat_emb),  s[n,f] = c'/sqrt(c'^2*rowvar(fe)[f]+1),
  c' = (Z - rowmean(Z)) / sqrt(eps*(rowvar(Z)+eps))   (eps-fold, no eps bcast)

Scores/softmax run transposed [k, n]; per-head score maps are single-scalar
tensor_scalar ops with the DAG mask folded into the exp's per-partition bias.

v5 (after v4 trace analysis, v4 = 35.8us):
 - attention numerator accumulates in [n, e] layout (wpre^T stationary, UV
   slices moving): softmax normalization becomes ONE broadcast tensor_tensor
   against rz4 [n,h] -- kills the transpose+bo4-matmul+copy rzb chain and one
   PSUM bank.
 - s-computation (sq -> v*c'^2+1 -> ln -> exp -> s) runs once on the full
   [128,256] transposed tile instead of per 128-chunk: half the chain hops,
   fewer fixed op overheads.
 - 4 DMA kicks ordered by need: pack1a (fe/label/q-chain), Z, pack2
   (A-mask/Wv), pack3 (W1/W2/Wo).
 - label row chain fully bf16 (g1row/beta1row as bf16 rows).
 - all matmuls/transposes bf16; no gpsimd; no bf16 TS subtract; ACT-free
   tail after the exps so the Gelu table load hides in the MM stream.
"""
import numpy as np
from contextlib import ExitStack

import concourse.bass as bass
import concourse.tile as tile
from concourse import bacc, mybir
from concourse.bass_utils import run_bass_kernel_spmd

F32 = mybir.dt.float32
BF16 = mybir.dt.bfloat16
AF = mybir.ActivationFunctionType
OP = mybir.AluOpType
AX = mybir.AxisListType

N, FD, E, H, DK, SEQ = 1024, 256, 128, 4, 32, 257
NCORES = 8
NP = N // NCORES
EPS = 1e-5
ISQ = float(1.0 / np.sqrt(DK))
LOG1P9 = float(np.log1p(1e-9))

# pack1a (bf16): earliest-needed.  vp is byte-punned fp32.
P1 = {"fe0": 0, "fe1": 128, "vp": 256, "vrow": 284, "g1rowB": 540,
      "b1rowB": 668, "idB": 796, "wkT": 924, "wq": 1052, "hm4": 1180}
P1_W = 1184
VCOL = {"labT": 0, "bq": 1, "bv": 2, "bo": 3, "b1a": 4, "b1b": 5, "b2": 6,
        "g1": 7, "beta1": 8, "g2": 9, "alpha": 10, "ceps": 11, "ceps2": 12,
        "clog": 13}
VPACK_W = 14
# pack2 (bf16): mask + Wv
P2 = {"apk": 0, "wv": 520, "beta1b": 648, "beta2b": 649}
APACK_W = 519
P2_W = 652
# pack3 (bf16): FFN/out weights
P3 = {"w1": 0, "w2": 256, "wo": 512}
P3_W = 640


def _body(tc, d, out_ap):
    nc = tc.nc
    ctx = ExitStack()
    with ctx:
        cp = ctx.enter_context(tc.tile_pool(name="cp", bufs=1))
        wp = ctx.enter_context(tc.tile_pool(name="wp", bufs=1))
        # PSUM pools: bank budget is 8.  t(2) + m(3) + pz(1) + on(1) = 7
        ps_t = ctx.enter_context(tc.tile_pool(name="ps_t", bufs=2, space="PSUM"))
        ps_m = ctx.enter_context(tc.tile_pool(name="ps_m", bufs=4, space="PSUM"))
        ps_z = ctx.enter_context(tc.tile_pool(name="ps_z", bufs=1, space="PSUM"))
        ps_o = ctx.enter_context(tc.tile_pool(name="ps_o", bufs=1, space="PSUM"))

        def sb(name, shape, pool=cp, dt=F32):
            return pool.tile(list(shape), dt, tag=name, name=name)

        def pt(name, shape, dt=F32, pool=ps_m, tag=None):
            return pool.tile(list(shape), dt, tag=(tag or "m"), name=name)

        # ---------------- loads (4 kicks, priority order) ----------------
        pk1 = sb("pk1", [128, P1_W], dt=BF16)
        nc.sync.dma_start(pk1[:], d["pack1"])
        pk2 = sb("pk2", [128, P2_W], dt=BF16)
        nc.sync.dma_start(pk2[:], d["pack2"])
        zb = sb("zb", [NP, FD], dt=BF16)
        nc.sync.dma_start(zb[:], d["Z"])
        pk3 = sb("pk3", [128, P3_W], dt=BF16)
        nc.sync.dma_start(pk3[:], d["pack3"])

        def W1s(name, w=128):
            return pk1[:, P1[name]:P1[name] + w]

        def W2s(name, w=128):
            return pk2[:, P2[name]:P2[name] + w]

        def W3s(name, w=128):
            return pk3[:, P3[name]:P3[name] + w]

        vp = pk1[:, P1["vp"]:P1["vp"] + 2 * VPACK_W].bitcast(F32)

        def V(name):
            return vp[:, VCOL[name]:VCOL[name] + 1]

        vrow = pk1[0:1, P1["vrow"]:P1["vrow"] + 2 * E].bitcast(F32)
        g1rowB = pk1[0:1, P1["g1rowB"]:P1["g1rowB"] + E]
        b1rowB = pk1[0:1, P1["b1rowB"]:P1["b1rowB"] + E]
        apk = pk2[:, P2["apk"]:P2["apk"] + APACK_W]
        idB = W1s("idB")
        g1c = V("g1")

        onescol = sb("onescol", [128, 1], dt=BF16)
        nc.vector.memset(onescol[:], 1.0)
        ones1 = sb("ones1", [1, 128], dt=BF16)
        nc.vector.memset(ones1[:], 1.0)

        # ---------------- label/q chain (gates the score coeffs) -----------
        stL = sb("stL", [1, 6], wp)
        nc.vector.bn_stats(stL[:], vrow)
        agL = sb("agL", [1, 2], wp)
        nc.vector.bn_aggr(agL[:], stL[:])
        lnL = sb("lnL", [1, 1], wp)
        nc.scalar.activation(lnL[:], agL[:, 1:2], AF.Ln,
                             bias=vp[0:1, VCOL["ceps"]:VCOL["ceps"] + 1])
        rstdL = sb("rstdL", [1, 1], wp)
        nc.scalar.activation(rstdL[:], lnL[:], AF.Exp, scale=-0.5)
        xl0rb = sb("xl0rb", [1, E], dt=BF16)
        nc.vector.tensor_scalar(out=xl0rb[:], in0=vrow, scalar1=agL[0:1, 0:1],
                                scalar2=rstdL[0:1, 0:1], op0=OP.subtract,
                                op1=OP.mult)
        dcolrb = sb("dcolrb", [1, E], dt=BF16)
        nc.vector.tensor_tensor(out=dcolrb[:], in0=xl0rb[:], in1=g1rowB,
                                op=OP.mult)
        xlastrb = sb("xlastrb", [1, E], dt=BF16)
        nc.vector.tensor_tensor(out=xlastrb[:], in0=dcolrb[:], in1=b1rowB,
                                op=OP.add)
        p_dc = pt("p_dc", [E, 1], dt=BF16, pool=ps_t, tag="t")
        nc.tensor.transpose(p_dc[:], dcolrb[:], idB[0:1, 0:1])
        p_xl = pt("p_xl", [E, 1], dt=BF16, pool=ps_t, tag="t")
        nc.tensor.transpose(p_xl[:], xlastrb[:], idB[0:1, 0:1])
        dcolb = sb("dcolb", [E, 1], dt=BF16)
        nc.scalar.copy(dcolb[:], p_dc[:])
        xlastTb = sb("xlastTb", [E, 1], dt=BF16)
        nc.scalar.copy(xlastTb[:], p_xl[:])
        xlastF = sb("xlastF", [E, 1], wp)
        nc.scalar.copy(xlastF[:], p_xl[:])

        p_q = pt("p_q", [128, 1])
        nc.tensor.matmul(p_q[:], W1s("wq"), xlastTb[:], start=True, stop=True)
        qcol = sb("qcol", [E, 1], wp)
        nc.vector.tensor_scalar_add(out=qcol[:], in0=p_q[:], scalar1=V("bq"))
        qm = sb("qm", [E, H], dt=BF16)
        nc.vector.tensor_scalar_mul(out=qm[:], in0=W1s("hm4", 4),
                                    scalar1=qcol[:, 0:1])
        p_tmp = pt("p_tmp", [128, H])
        nc.tensor.matmul(p_tmp[:], W1s("wkT"), qm[:], start=True, stop=True)
        th = sb("th", [E, H], dt=BF16)      # ISQ * (Wk @ qm); g1 lives in ut
        nc.vector.tensor_scalar_mul(out=th[:], in0=p_tmp[:], scalar1=ISQ)

        # ---------------- feat_emb stats: g1*u^T chunks, var cols ----------
        uts, vcols = [], []
        for i in range(2):
            fe = W1s("fe0") if i == 0 else W1s("fe1")
            st = sb(f"st{i}", [128, 6], wp)
            nc.vector.bn_stats(st[:], fe)
            ag = sb(f"ag{i}", [128, 2])
            nc.vector.bn_aggr(ag[:], st[:])
            negm = sb(f"negm{i}", [128, 1], wp)
            nc.vector.tensor_scalar_mul(out=negm[:], in0=ag[:, 0:1], scalar1=-1.0)
            u = sb(f"u{i}", [128, E], dt=BF16)
            nc.vector.tensor_scalar_add(out=u[:], in0=fe, scalar1=negm[:, 0:1])
            p_ut = pt(f"p_ut{i}", [128, 128], dt=BF16, pool=ps_t, tag="t")
            nc.tensor.transpose(p_ut[:], u[:], idB)
            ut = sb(f"ut{i}", [128, 128], dt=BF16)   # = (g1 * u)^T
            nc.vector.tensor_scalar_mul(out=ut[:], in0=p_ut[:], scalar1=g1c)
            uts.append(ut)
            vcols.append(ag[:, 1:2])

        # a' columns (score coefficients), both chunks in one tile
        p_a8 = pt("p_a8", [128, 2 * H])
        nc.tensor.matmul(p_a8[:, 0:H], uts[0][:], th[:], start=True, stop=True,
                         skip_group_check=True)
        nc.tensor.matmul(p_a8[:, H:2 * H], uts[1][:], th[:], start=True,
                         stop=True, skip_group_check=True)
        ac8b = sb("ac8b", [128, 2 * H], dt=BF16)
        nc.scalar.copy(ac8b[:], p_a8[:])

        # ===== z-chain + transposed s computation (early: gates main) =====
        stZ = sb("stZ", [NP, 6], wp); nc.vector.bn_stats(stZ[:], zb[:])
        agZ = sb("agZ", [NP, 2], wp); nc.vector.bn_aggr(agZ[:], stZ[:])
        lnE = sb("lnE", [NP, 1], wp)
        nc.scalar.activation(lnE[:], agZ[:, 1:2], AF.Ln,
                             bias=vp[:, VCOL["ceps2"]:VCOL["ceps2"] + 1],
                             scale=EPS)
        rEcol = sb("rEcol", [NP, 1], wp)
        nc.scalar.activation(rEcol[:], lnE[:], AF.Exp, scale=-0.5)
        nmZ = sb("nmZ", [NP, 1], wp)
        nc.vector.tensor_scalar_mul(out=nmZ[:], in0=agZ[:, 0:1], scalar1=-1.0)
        nmrE = sb("nmrE", [NP, 1], wp)
        nc.vector.tensor_tensor(out=nmrE[:], in0=nmZ[:], in1=rEcol[:], op=OP.mult)
        zn = sb("zn", [NP, FD], dt=BF16)
        nc.vector.tensor_scalar(out=zn[:], in0=zb[:], scalar1=rEcol[:, 0:1],
                                scalar2=nmrE[:, 0:1], op0=OP.mult, op1=OP.add)

        # full-width transposed s computation
        p_znT = pt("p_znT", [128, 256], dt=BF16, pool=ps_t, tag="t")
        for i in range(2):
            nc.tensor.matmul(p_znT[:, 128 * i:128 * (i + 1)],
                             zn[:, 128 * i:128 * (i + 1)], idB,
                             is_transpose=True, start=True, stop=True,
                             skip_group_check=True)
        sq = wp.tile([128, 256], BF16, tag="sq", name="sq")
        nc.scalar.activation(sq[:], p_znT[:], AF.Square)
        vc2 = sb("vc2", [128, 2], wp)
        nc.vector.tensor_copy(out=vc2[:, 0:1], in_=vcols[0])
        nc.vector.tensor_copy(out=vc2[:, 1:2], in_=vcols[1])
        w1t = wp.tile([128, 2, 128], BF16, tag="w1t", name="w1t")
        nc.vector.tensor_tensor(
            out=w1t[:], in0=sq[:].rearrange("p (g i) -> p g i", g=2),
            in1=vc2[:].unsqueeze(2).broadcast_to((128, 2, 128)), op=OP.mult)
        onesF = sb("onesF", [128, 1])
        nc.vector.memset(onesF[:], 1.0)
        lnt = wp.tile([128, 256], F32, tag="lnt", name="lnt")
        nc.scalar.activation(lnt[:], w1t[:], AF.Ln, bias=onesF[:, 0:1])
        rst = wp.tile([128, 256], BF16, tag="rst", name="rst")
        nc.scalar.activation(rst[:], lnt[:], AF.Exp, scale=-0.5)
        sT = wp.tile([128, 256], BF16, tag="sT", name="sT")
        nc.vector.tensor_tensor(out=sT[:], in0=p_znT[:], in1=rst[:], op=OP.mult)


        # ---------------- mask: c_max + mask columns (no gpsimd) -----------
        mred = sb("mred", [128, 1], wp)
        nc.vector.tensor_reduce(out=mred[:], in_=apk, op=OP.max, axis=AX.X,
                                apply_absolute_value=True)
        mrb = sb("mrb", [128, 1], dt=BF16)
        nc.vector.tensor_copy(out=mrb[:], in_=mred[:])
        p_mr = pt("p_mr", [1, 128], dt=BF16, pool=ps_t, tag="t")
        nc.tensor.transpose(p_mr[:], mrb[:], idB)
        cm1 = sb("cm1", [1, 1], wp)
        nc.vector.tensor_reduce(out=cm1[:], in_=p_mr[:], op=OP.max, axis=AX.X)
        rec1 = sb("rec1", [1, 1], wp)
        nc.vector.reciprocal(rec1[:], cm1[:])
        ge1 = sb("ge1", [1, 1], wp)
        nc.vector.tensor_scalar(out=ge1[:], in0=cm1[:], scalar1=1e-6,
                                scalar2=None, op0=OP.is_gt)
        f1c = sb("f1c", [1, 1], wp)
        nc.vector.tensor_scalar_add(out=f1c[:], in0=rec1[:], scalar1=-1.0)
        nc.vector.tensor_tensor(out=f1c[:], in0=ge1[:], in1=f1c[:], op=OP.mult)
        nc.vector.tensor_scalar_add(out=f1c[:], in0=f1c[:], scalar1=1.0)
        gof1 = sb("gof1", [1, 1], wp)
        nc.vector.tensor_scalar(out=gof1[:], in0=ge1[:], scalar1=-1e-3,
                                scalar2=1e-3 + 1e-9, op0=OP.mult, op1=OP.add)
        # one rank-1 broadcast for [f, g, alpha]
        fga = sb("fga", [1, 3], dt=BF16)
        nc.vector.tensor_copy(out=fga[:, 0:1], in_=f1c[:])
        nc.vector.tensor_copy(out=fga[:, 1:2], in_=gof1[:])
        nc.vector.tensor_copy(out=fga[:, 2:3],
                              in_=vp[0:1, VCOL["alpha"]:VCOL["alpha"] + 1])
        p_fga = pt("p_fga", [128, 3])
        nc.tensor.matmul(p_fga[:], ones1[:], fga[:], start=True, stop=True)
        fgacol = sb("fgacol", [128, 3])
        nc.vector.tensor_copy(out=fgacol[:], in_=p_fga[:])
        fcol, gcol, alcol = fgacol[:, 0:1], fgacol[:, 1:2], fgacol[:, 2:3]
        aab = sb("aab", [128, 2], wp)
        nc.scalar.activation(aab[:], apk[:, 517:519], AF.Abs)
        mkc = sb("mkc", [128, 2])
        nc.scalar.activation(mkc[:], aab[:], AF.Ln, bias=gcol, scale=fcol)

        # ---------------- label-score consts + label V row ----------------
        p_cr = pt("p_cr", [1, H])
        nc.tensor.matmul(p_cr[:], dcolb[:], th[:], start=True, stop=True)
        ecrow = sb("ecrow", [1, H], dt=BF16)
        nc.scalar.activation(ecrow[:], p_cr[:], AF.Exp,
                             bias=vp[0:1, VCOL["clog"]:VCOL["clog"] + 1])
        p_vdr = pt("p_vdr", [1, E])
        nc.tensor.matmul(p_vdr[:], dcolb[:], W2s("wv"), start=True, stop=True)
        ulcrow = sb("ulcrow", [1, E], dt=BF16)
        nc.vector.tensor_tensor(
            out=ulcrow[:].rearrange("p (g i) -> p g i", g=H),
            in0=p_vdr[:].rearrange("p (g i) -> p g i", g=H),
            in1=ecrow[:].unsqueeze(2).broadcast_to((1, H, 32)), op=OP.mult)

        # ---------------- UV chunks (g1 already in ut) ----------------
        uvs = []
        for i in range(2):
            p_uv = pt(f"p_uv{i}", [128, 128])
            nc.tensor.matmul(p_uv[:], uts[i][:], W2s("wv"), start=True, stop=True)
            uv = sb(f"uv{i}", [128, E], dt=BF16)
            nc.scalar.copy(uv[:], p_uv[:])
            uvs.append(uv)

        # ================= main phase =================
        pz4 = ps_z.tile([128, H], F32, tag="pz", name="pz4")
        p_on = ps_o.tile([128, 128], F32, tag="on", name="p_on")
        # label-position rank-1 init of the accumulators
        nc.tensor.matmul(pz4[:], ones1[:], ecrow[:], start=True, stop=False,
                         skip_group_check=True)
        nc.tensor.matmul(p_on[:], ones1[:], ulcrow[:], start=True, stop=False,
                         skip_group_check=True)
        # batched score map: one TT for all (chunk, head) pairs
        scTbig = wp.tile([128, 2, H, 128], BF16, tag="scTbig", name="scTbig")
        nc.vector.tensor_tensor(
            out=scTbig[:],
            in0=sT[:].rearrange("p (g i) -> p g i", g=2).unsqueeze(2)
                .broadcast_to((128, 2, H, 128)),
            in1=ac8b[:].rearrange("p (g h) -> p g h", g=2).unsqueeze(3)
                .broadcast_to((128, 2, H, 128)),
            op=OP.mult)
        eTbig = wp.tile([128, 2, H, 128], BF16, tag="eTbig", name="eTbig")
        for i in range(2):
            nc.scalar.activation(eTbig[:, i], scTbig[:, i], AF.Exp,
                                 bias=mkc[:, i:i + 1])
        wpreTbig = wp.tile([128, 2, H, 128], BF16, tag="wpreTbig",
                           name="wpreTbig")
        nc.vector.tensor_tensor(
            out=wpreTbig[:], in0=eTbig[:],
            in1=sT[:].rearrange("p (g i) -> p g i", g=2).unsqueeze(2)
                .broadcast_to((128, 2, H, 128)),
            op=OP.mult)
        # pz4 accums first: pz4 finishes before the numerator stream, so the
        # reciprocal overlaps the p_on matmuls.
        for i in range(2):
            for h in range(H):
                nc.tensor.matmul(pz4[:, h:h + 1], eTbig[:, i, h, :], onescol[:],
                                 start=False, stop=(i == 1 and h == H - 1),
                                 skip_group_check=True)
        for i in range(2):
            for h in range(H):
                nc.tensor.matmul(p_on[:, 32 * h:32 * (h + 1)],
                                 wpreTbig[:, i, h, :],
                                 uvs[i][:, 32 * h:32 * (h + 1)], start=False,
                                 stop=(i == 1 and h == H - 1),
                                 skip_group_check=True)

        # vccol folded into bo via Wo;  rescol for the final combine
        p_vc = pt("p_vc", [128, 1])
        nc.tensor.matmul(p_vc[:], W2s("wv"), W2s("beta1b", 1), start=True,
                         stop=True)
        vccol = sb("vccol", [E, 1], dt=BF16)
        nc.vector.tensor_scalar_add(out=vccol[:], in0=p_vc[:], scalar1=V("bv"))
        p_wovc = pt("p_wovc", [128, 1])
        nc.tensor.matmul(p_wovc[:], W3s("wo"), vccol[:], start=True, stop=True)
        boful = sb("boful", [E, 1], wp)
        nc.vector.tensor_scalar_add(out=boful[:], in0=p_wovc[:], scalar1=V("bo"))
        rescol = sb("rescol", [E, 1], wp)
        nc.vector.tensor_tensor(out=rescol[:], in0=boful[:], in1=V("b2"), op=OP.add)
        nc.vector.tensor_scalar(out=rescol[:], in0=rescol[:], scalar1=alcol,
                                scalar2=xlastF[:, 0:1], op0=OP.mult, op1=OP.add)

        # FFN bias consts + alpha-scaled W2
        b1ps = []
        for i, bn in enumerate(("b1a", "b1b")):
            p_b1 = pt(f"p_b1{i}", [128, 1])
            nc.tensor.matmul(p_b1[:], W3s("w1", 256)[:, 128 * i:128 * (i + 1)],
                             W2s("beta2b", 1), start=True, stop=True)
            b1p = sb(f"b1p{i}", [128, 1])
            nc.vector.tensor_scalar_add(out=b1p[:], in0=p_b1[:], scalar1=V(bn))
            b1ps.append(b1p)
        w2p = sb("w2p", [E, 2 * E], dt=BF16)
        nc.vector.tensor_scalar_mul(out=w2p[:], in0=W3s("w2", 256), scalar1=alcol)

        # normalize in [n, e]: oa = p_on * (1/Z)[n, h(e)]
        rz4 = sb("rz4", [128, H], wp)
        nc.vector.reciprocal(rz4[:], pz4[:])
        oab = sb("oab", [NP, E], dt=BF16)
        nc.vector.tensor_tensor(
            out=oab[:].rearrange("p (g i) -> p g i", g=H),
            in0=p_on[:].rearrange("p (g i) -> p g i", g=H),
            in1=rz4[:].unsqueeze(2).broadcast_to((128, H, 32)), op=OP.mult)
        p_oaT = pt("p_oaT", [E, NP], dt=BF16, pool=ps_t, tag="t")
        nc.tensor.transpose(p_oaT[:], oab[:], idB)
        oaT = sb("oaT", [E, 128], dt=BF16)
        nc.vector.tensor_copy(out=oaT[:], in_=p_oaT[:])

        # Wo
        p_wo = pt("p_wo", [128, 128])
        nc.tensor.matmul(p_wo[:], W3s("wo"), oaT[:], start=True, stop=True)
        ooT = sb("ooT", [E, 128], dt=BF16)
        nc.vector.tensor_scalar_add(out=ooT[:], in0=p_wo[:], scalar1=boful[:, 0:1])
        resid2 = sb("resid2", [E, 128], wp)
        nc.vector.tensor_scalar(out=resid2[:], in0=p_wo[:], scalar1=alcol,
                                scalar2=rescol[:, 0:1], op0=OP.mult, op1=OP.add)

        # LN over emb (stats in [n, e] layout)
        p_tn = pt("p_tn", [128, 128], dt=BF16, pool=ps_t, tag="t")
        nc.tensor.transpose(p_tn[:], ooT[:], idB)
        stO = sb("stO", [128, 6], wp); nc.vector.bn_stats(stO[:], p_tn[:])
        agO = sb("agO", [128, 2], wp); nc.vector.bn_aggr(agO[:], stO[:])
        vO = sb("vO", [128, 1], wp)
        nc.vector.tensor_scalar_add(out=vO[:], in0=agO[:, 1:2], scalar1=EPS)
        nmO = sb("nmO", [128, 1], wp)
        nc.vector.tensor_scalar_mul(out=nmO[:], in0=agO[:, 0:1], scalar1=-1.0)
        lnO = sb("lnO", [128, 1], wp)
        nc.scalar.activation(lnO[:], vO[:], AF.Ln)
        rstdO = sb("rstdO", [128, 1], wp)
        nc.scalar.activation(rstdO[:], lnO[:], AF.Exp, scale=-0.5)
        hpre = sb("hpre", [128, 128], dt=BF16)
        nc.vector.tensor_scalar(out=hpre[:], in0=p_tn[:], scalar1=nmO[:, 0:1],
                                scalar2=rstdO[:, 0:1], op0=OP.add, op1=OP.mult)
        p_ht = pt("p_ht", [128, 128], dt=BF16, pool=ps_t, tag="t")
        nc.tensor.transpose(p_ht[:], hpre[:], idB)
        hT = sb("hT", [128, 128], dt=BF16)   # g2 folded here
        nc.vector.tensor_scalar_mul(out=hT[:], in0=p_ht[:], scalar1=V("g2"))

        # FFN (both halves share one PSUM bank)
        p_f1 = pt("p_f1", [128, 256])
        nc.tensor.matmul(p_f1[:, 0:128], W3s("w1", 256)[:, 0:128], hT[:],
                         start=True, stop=True, skip_group_check=True)
        nc.tensor.matmul(p_f1[:, 128:256], W3s("w1", 256)[:, 128:256], hT[:],
                         start=True, stop=True, skip_group_check=True)
        gt = wp.tile([128, 2, 128], BF16, tag="gt", name="gt")
        nc.scalar.activation(gt[:, 0, :], p_f1[:, 0:128], AF.Gelu,
                             bias=b1ps[0][:, 0:1])
        nc.scalar.activation(gt[:, 1, :], p_f1[:, 128:256], AF.Gelu,
                             bias=b1ps[1][:, 0:1])
        p_y = pt("p_y", [128, 128])
        nc.tensor.matmul(p_y[:], w2p[:, 0:128], gt[:, 0, :], start=True, stop=False)
        nc.tensor.matmul(p_y[:], w2p[:, 128:256], gt[:, 1, :], start=False,
                         stop=True)

        # final combine [e, n]; host transposes
        zfT = sb("zfT", [128, 128], wp)
        nc.vector.tensor_tensor(out=zfT[:], in0=p_y[:], in1=resid2[:], op=OP.add)
        nc.sync.dma_start(out_ap, zfT[:])


_CACHE = {}


def _restrict_act_tables():
    """Limit the act-table-load pass to two sets so every non-Gelu activation
    (abs/copy/exp/identity/ln/square) resolves to one table and Gelu to the
    other -- avoids ~1.3us table reloads from per-function set churn."""
    import concourse.hw_specs as hws
    import concourse.bacc as bacc_mod
    orig = hws.get_activation_tables

    def patched(arch):
        t = orig(arch)
        keep = {}
        n_good = 0
        for name, fns in t.items():
            fnames = {f.name for f in fns}
            good = ("Ln" in fnames and "Exp" in fnames) or "Gelu" in fnames
            keep[name] = fns if good else set()   # keep positions for set ids
            n_good += bool(good)
        assert n_good >= 2, f"unexpected act table sets: {list(t)}"
        return keep

    bacc_mod.get_activation_tables = patched


def _get_nc():
    if "nc" in _CACHE:
        return _CACHE["nc"]
    _restrict_act_tables()
    nc = bacc.Bacc("TRN2", target_bir_lowering=False, debug=False,
                   num_devices=NCORES)
    d = {}
    for name, shape, dt in (("pack1", (128, P1_W), BF16),
                            ("Z", (NP, FD), BF16),
                            ("pack2", (128, P2_W), BF16),
                            ("pack3", (128, P3_W), BF16)):
        d[name] = nc.dram_tensor(name, list(shape), dt, kind="ExternalInput").ap()
    out_ap = nc.dram_tensor("out", [E, NP], F32, kind="ExternalOutput").ap()
    with tile.TileContext(nc) as tc:
        _body(tc, d, out_ap)
    nc.compile()
    _CACHE["nc"] = nc
    return nc


def _in_maps(inputs):
    import ml_dtypes
    bf = ml_dtypes.bfloat16
    a = {k: np.ascontiguousarray(np.asarray(v, dtype=np.float32))
         for k, v in inputs.items()}

    pack1 = np.zeros((128, P1_W), bf)
    pack1[:, P1["fe0"]:P1["fe0"] + 128] = a["feat_emb"][0:128].astype(bf)
    pack1[:, P1["fe1"]:P1["fe1"] + 128] = a["feat_emb"][128:256].astype(bf)
    vpack = np.zeros((128, VPACK_W), np.float32)
    vpack[:, VCOL["labT"]] = a["label_token"].reshape(E)
    for nm in ("bq", "bv", "bo", "b2", "g1", "beta1", "g2"):
        vpack[:, VCOL[nm]] = a[nm]
    vpack[:, VCOL["b1a"]] = a["b1"][0:128]
    vpack[:, VCOL["b1b"]] = a["b1"][128:256]
    vpack[0, VCOL["alpha"]] = float(np.asarray(a["alpha_res"]).reshape(-1)[0])
    vpack[:, VCOL["ceps"]] = EPS
    vpack[:, VCOL["ceps2"]] = EPS * EPS
    vpack[:, VCOL["clog"]] = LOG1P9
    pack1[:, P1["vp"]:P1["vp"] + 2 * VPACK_W] = np.ascontiguousarray(vpack).view(bf)
    pack1[0, P1["vrow"]:P1["vrow"] + 2 * E] = np.ascontiguousarray(
        a["label_token"].reshape(1, E)).view(bf)[0]
    pack1[0, P1["g1rowB"]:P1["g1rowB"] + E] = a["g1"].astype(bf)
    pack1[0, P1["b1rowB"]:P1["b1rowB"] + E] = a["beta1"].astype(bf)
    pack1[:, P1["idB"]:P1["idB"] + 128] = np.eye(128, dtype=np.float32).astype(bf)
    pack1[:, P1["wkT"]:P1["wkT"] + 128] = a["Wk"].T.astype(bf)
    pack1[:, P1["wq"]:P1["wq"] + 128] = a["Wq"].astype(bf)
    pack1[:, P1["hm4"]:P1["hm4"] + 4] = np.repeat(
        np.eye(4, dtype=np.float32), 32, axis=0).astype(bf)

    A = a["A_no_diag"]
    apack = np.zeros((128, APACK_W), np.float32)
    apack[:, 0:SEQ] = A[0:128]
    apack[:, SEQ:2 * SEQ] = A[128:256]
    apack[:, 514] = A[256, 0:128]
    apack[:, 515] = A[256, 128:256]
    apack[0, 516] = A[256, 256]
    apack[:, 517] = A[0:128, 256]
    apack[:, 518] = A[128:256, 256]
    pack2 = np.zeros((128, P2_W), bf)
    pack2[:, P2["apk"]:P2["apk"] + APACK_W] = apack.astype(bf)
    pack2[:, P2["wv"]:P2["wv"] + 128] = a["Wv"].astype(bf)
    pack2[:, P2["beta1b"]] = a["beta1"].astype(bf)
    pack2[:, P2["beta2b"]] = a["beta2"].astype(bf)

    pack3 = np.zeros((128, P3_W), bf)
    pack3[:, P3["w1"]:P3["w1"] + 256] = a["W1"].astype(bf)
    pack3[:, P3["w2"]:P3["w2"] + 256] = np.concatenate(
        [a["W2"][0:128], a["W2"][128:256]], axis=1).astype(bf)
    pack3[:, P3["wo"]:P3["wo"] + 128] = a["Wo"].astype(bf)

    maps = []
    for c in range(NCORES):
        m = {"pack1": pack1, "pack2": pack2, "pack3": pack3,
             "Z": np.ascontiguousarray(a["Z"][c * NP:(c + 1) * NP]).astype(bf)}
        maps.append(m)
    return maps


def run(inputs, trace=False):
    nc = _get_nc()
    res = run_bass_kernel_spmd(nc, _in_maps(inputs), core_ids=list(range(NCORES)),
                               trace=trace)
    out = np.concatenate([res.results[c]["out"].T for c in range(NCORES)], axis=0)
    return np.ascontiguousarray(out.astype(np.float32)), res


def kernel(**inputs):
    out, _ = run(inputs, trace=False)
    return out


# revision 22
# speedup vs baseline: 1.1392x; 1.1392x over previous
"""Trainium2 Bass kernel for nn_CausalFeatureTransformer (v5).

Only the last sequence position (label token) of the reference output is needed,
so the per-sample transformer collapses (see derivation in v1):

  X_norm[n,f,:] = s[n,f]*u[f,:]*g1 + beta1  (f<256),  X_norm[n,256,:] = xlast
  u = feat_emb - rowmean(feat_emb),  s[n,f] = c'/sqrt(c'^2*rowvar(fe)[f]+1),
  c' = (Z - rowmean(Z)) / sqrt(eps*(rowvar(Z)+eps))   (eps-fold, no eps bcast)

Scores/softmax run transposed [k, n]; per-head score maps are single-scalar
tensor_scalar ops with the DAG mask folded into the exp's per-partition bias.

v5 (after v4 trace analysis, v4 = 35.8us):
 - attention numerator accumulates in [n, e] layout (wpre^T stationary, UV
   slices moving): softmax normalization becomes ONE broadcast tensor_tensor
   against rz4 [n,h] -- kills the transpose+bo4-matmul+copy rzb chain and one
   PSUM bank.
 - s-computation (sq -> v*c'^2+1 -> ln -> exp -> s) runs once on the full
   [128,256] transposed tile instead of per 128-chunk: half the chain hops,
   fewer fixed op overheads.
 - 4 DMA kicks ordered by need: pack1a (fe/label/q-chain), Z, pack2
   (A-mask/Wv), pack3 (W1/W2/Wo).
 - label row chain fully bf16 (g1row/beta1row as bf16 rows).
 - all matmuls/transposes bf16; no gpsimd; no bf16 TS subtract; ACT-free
   tail after the exps so the Gelu table load hides in the MM stream.
"""
import numpy as np
from contextlib import ExitStack

import concourse.bass as bass
import concourse.tile as tile
from concourse import bacc, mybir
from concourse.bass_utils import run_bass_kernel_spmd

F32 = mybir.dt.float32
BF16 = mybir.dt.bfloat16
AF = mybir.ActivationFunctionType
OP = mybir.AluOpType
AX = mybir.AxisListType

N, FD, E, H, DK, SEQ = 1024, 256, 128, 4, 32, 257
NCORES = 8
NP = N // NCORES
EPS = 1e-5
ISQ = float(1.0 / np.sqrt(DK))
LOG1P9 = float(np.log1p(1e-9))

# pack1a (bf16): earliest-needed.  vp is byte-punned fp32.
P1 = {"fe0": 0, "fe1": 128, "vp": 256, "vrow": 284, "g1rowB": 540,
      "b1rowB": 668, "idB": 796, "wkT": 924, "wq": 1052, "hm4": 1180}
P1_W = 1184
VCOL = {"labT": 0, "bq": 1, "bv": 2, "bo": 3, "b1a": 4, "b1b": 5, "b2": 6,
        "g1": 7, "beta1": 8, "g2": 9, "alpha": 10, "ceps": 11, "ceps2": 12,
        "clog": 13}
VPACK_W = 14
# pack2 (bf16): mask + Wv
P2 = {"apk": 0, "wv": 520, "beta1b": 648, "beta2b": 649}
APACK_W = 519
P2_W = 652
# pack3 (bf16): FFN/out weights
P3 = {"w1": 0, "w2": 256, "wo": 512}
P3_W = 640


def _body(tc, d, out_ap):
    nc = tc.nc
    ctx = ExitStack()
    with ctx:
        cp = ctx.enter_context(tc.tile_pool(name="cp", bufs=1))
        wp = ctx.enter_context(tc.tile_pool(name="wp", bufs=1))
        # PSUM pools: bank budget is 8.  t(2) + m(3) + pz(1) + on(1) = 7
        ps_t = ctx.enter_context(tc.tile_pool(name="ps_t", bufs=2, space="PSUM"))
        ps_m = ctx.enter_context(tc.tile_pool(name="ps_m", bufs=4, space="PSUM"))
        ps_z = ctx.enter_context(tc.tile_pool(name="ps_z", bufs=1, space="PSUM"))
        ps_o = ctx.enter_context(tc.tile_pool(name="ps_o", bufs=1, space="PSUM"))

        def sb(name, shape, pool=cp, dt=F32):
            return pool.tile(list(shape), dt, tag=name, name=name)

        def pt(name, shape, dt=F32, pool=ps_m, tag=None):
            return pool.tile(list(shape), dt, tag=(tag or "m"), name=name)

        # ---------------- loads (4 kicks, priority order) ----------------
        pk1 = sb("pk1", [128, P1_W], dt=BF16)
        nc.sync.dma_start(pk1[:], d["pack1"])
        pk2 = sb("pk2", [128, P2_W], dt=BF16)
        nc.sync.dma_start(pk2[:], d["pack2"])
        zb = sb("zb", [NP, FD], dt=BF16)
        nc.sync.dma_start(zb[:], d["Z"])
        pk3 = sb("pk3", [128, P3_W], dt=BF16)
        nc.sync.dma_start(pk3[:], d["pack3"])

        def W1s(name, w=128):
            return pk1[:, P1[name]:P1[name] + w]

        def W2s(name, w=128):
            return pk2[:, P2[name]:P2[name] + w]

        def W3s(name, w=128):
            return pk3[:, P3[name]:P3[name] + w]

        vp = pk1[:, P1["vp"]:P1["vp"] + 2 * VPACK_W].bitcast(F32)

        def V(name):
            return vp[:, VCOL[name]:VCOL[name] + 1]

        vrow = pk1[0:1, P1["vrow"]:P1["vrow"] + 2 * E].bitcast(F32)
        g1rowB = pk1[0:1, P1["g1rowB"]:P1["g1rowB"] + E]
        b1rowB = pk1[0:1, P1["b1rowB"]:P1["b1rowB"] + E]
        apk = pk2[:, P2["apk"]:P2["apk"] + APACK_W]
        idB = W1s("idB")
        g1c = V("g1")

        onescol = sb("onescol", [128, 1], dt=BF16)
        nc.vector.memset(onescol[:], 1.0)
        ones1 = sb("ones1", [1, 128], dt=BF16)
        nc.vector.memset(ones1[:], 1.0)

        # HAM warm-up: ~24 back-to-back dummy matmuls (~3.4us of PE busy)
        # flip the PE clock gate to 8/8 (2.4 GHz) before real PE work starts;
        # later MM gaps stay under the ~3.4us MID re-throttle window.
        scr = sb("scr", [128, 128], dt=BF16)
        nc.vector.memset(scr[:], 0.0)
        p_warm = ps_o.tile([128, 128], F32, tag="on", name="p_warm")
        for _ in range(24):
            nc.tensor.matmul(p_warm[:], scr[:], scr[:], start=True, stop=True,
                             skip_group_check=True)

        # ---------------- label/q chain (gates the score coeffs) -----------
        stL = sb("stL", [1, 6], wp)
        nc.vector.bn_stats(stL[:], vrow)
        agL = sb("agL", [1, 2], wp)
        nc.vector.bn_aggr(agL[:], stL[:])
        lnL = sb("lnL", [1, 1], wp)
        nc.scalar.activation(lnL[:], agL[:, 1:2], AF.Ln,
                             bias=vp[0:1, VCOL["ceps"]:VCOL["ceps"] + 1])
        rstdL = sb("rstdL", [1, 1], wp)
        nc.scalar.activation(rstdL[:], lnL[:], AF.Exp, scale=-0.5)
        xl0rb = sb("xl0rb", [1, E], dt=BF16)
        nc.vector.tensor_scalar(out=xl0rb[:], in0=vrow, scalar1=agL[0:1, 0:1],
                                scalar2=rstdL[0:1, 0:1], op0=OP.subtract,
                                op1=OP.mult)
        dcolrb = sb("dcolrb", [1, E], dt=BF16)
        nc.vector.tensor_tensor(out=dcolrb[:], in0=xl0rb[:], in1=g1rowB,
                                op=OP.mult)
        xlastrb = sb("xlastrb", [1, E], dt=BF16)
        nc.vector.tensor_tensor(out=xlastrb[:], in0=dcolrb[:], in1=b1rowB,
                                op=OP.add)
        p_dc = pt("p_dc", [E, 1], dt=BF16, pool=ps_t, tag="t")
        nc.tensor.transpose(p_dc[:], dcolrb[:], idB[0:1, 0:1])
        p_xl = pt("p_xl", [E, 1], dt=BF16, pool=ps_t, tag="t")
        nc.tensor.transpose(p_xl[:], xlastrb[:], idB[0:1, 0:1])
        dcolb = sb("dcolb", [E, 1], dt=BF16)
        nc.vector.tensor_copy(out=dcolb[:], in_=p_dc[:])
        xlastTb = sb("xlastTb", [E, 1], dt=BF16)
        nc.vector.tensor_copy(out=xlastTb[:], in_=p_xl[:])
        xlastF = sb("xlastF", [E, 1], wp)
        nc.vector.tensor_copy(out=xlastF[:], in_=p_xl[:])

        p_q = pt("p_q", [128, 1])
        nc.tensor.matmul(p_q[:], W1s("wq"), xlastTb[:], start=True, stop=True)
        qcol = sb("qcol", [E, 1], wp)
        nc.vector.tensor_scalar_add(out=qcol[:], in0=p_q[:], scalar1=V("bq"))
        qm = sb("qm", [E, H], dt=BF16)
        nc.vector.tensor_scalar_mul(out=qm[:], in0=W1s("hm4", 4),
                                    scalar1=qcol[:, 0:1])
        p_tmp = pt("p_tmp", [128, H])
        nc.tensor.matmul(p_tmp[:], W1s("wkT"), qm[:], start=True, stop=True)
        th = sb("th", [E, H], dt=BF16)      # ISQ * (Wk @ qm); g1 lives in ut
        nc.vector.tensor_scalar_mul(out=th[:], in0=p_tmp[:], scalar1=ISQ)

        # ---------------- feat_emb stats: g1*u^T chunks, var cols ----------
        uts, vcols = [], []
        for i in range(2):
            fe = W1s("fe0") if i == 0 else W1s("fe1")
            st = sb(f"st{i}", [128, 6], wp)
            nc.vector.bn_stats(st[:], fe)
            ag = sb(f"ag{i}", [128, 2])
            nc.vector.bn_aggr(ag[:], st[:])
            negm = sb(f"negm{i}", [128, 1], wp)
            nc.vector.tensor_scalar_mul(out=negm[:], in0=ag[:, 0:1], scalar1=-1.0)
            u = sb(f"u{i}", [128, E], dt=BF16)
            nc.vector.tensor_scalar_add(out=u[:], in0=fe, scalar1=negm[:, 0:1])
            p_ut = pt(f"p_ut{i}", [128, 128], dt=BF16, pool=ps_t, tag="t")
            nc.tensor.transpose(p_ut[:], u[:], idB)
            ut = sb(f"ut{i}", [128, 128], dt=BF16)   # = (g1 * u)^T
            nc.vector.tensor_scalar_mul(out=ut[:], in0=p_ut[:], scalar1=g1c)
            uts.append(ut)
            vcols.append(ag[:, 1:2])

        # a' columns (score coefficients), both chunks in one tile
        p_a8 = pt("p_a8", [128, 2 * H])
        nc.tensor.matmul(p_a8[:, 0:H], uts[0][:], th[:], start=True, stop=True,
                         skip_group_check=True)
        nc.tensor.matmul(p_a8[:, H:2 * H], uts[1][:], th[:], start=True,
                         stop=True, skip_group_check=True)
        ac8b = sb("ac8b", [128, 2 * H], dt=BF16)
        nc.vector.tensor_copy(out=ac8b[:], in_=p_a8[:])

        # ===== z-chain + transposed s computation (early: gates main) =====
        stZ = sb("stZ", [NP, 6], wp); nc.vector.bn_stats(stZ[:], zb[:])
        agZ = sb("agZ", [NP, 2], wp); nc.vector.bn_aggr(agZ[:], stZ[:])
        lnE = sb("lnE", [NP, 1], wp)
        nc.scalar.activation(lnE[:], agZ[:, 1:2], AF.Ln,
                             bias=vp[:, VCOL["ceps2"]:VCOL["ceps2"] + 1],
                             scale=EPS)
        rEcol = sb("rEcol", [NP, 1], wp)
        nc.scalar.activation(rEcol[:], lnE[:], AF.Exp, scale=-0.5)
        nmZ = sb("nmZ", [NP, 1], wp)
        nc.vector.tensor_scalar_mul(out=nmZ[:], in0=agZ[:, 0:1], scalar1=-1.0)
        nmrE = sb("nmrE", [NP, 1], wp)
        nc.vector.tensor_tensor(out=nmrE[:], in0=nmZ[:], in1=rEcol[:], op=OP.mult)
        zn = sb("zn", [NP, FD], dt=BF16)
        nc.vector.tensor_scalar(out=zn[:], in0=zb[:], scalar1=rEcol[:, 0:1],
                                scalar2=nmrE[:, 0:1], op0=OP.mult, op1=OP.add)

        # full-width transposed s computation
        p_znT = pt("p_znT", [128, 256], dt=BF16, pool=ps_t, tag="t")
        for i in range(2):
            nc.tensor.matmul(p_znT[:, 128 * i:128 * (i + 1)],
                             zn[:, 128 * i:128 * (i + 1)], idB,
                             is_transpose=True, start=True, stop=True,
                             skip_group_check=True)
        sq = wp.tile([128, 256], BF16, tag="sq", name="sq")
        nc.scalar.activation(sq[:], p_znT[:], AF.Square)
        vc2 = sb("vc2", [128, 2], wp)
        nc.vector.tensor_copy(out=vc2[:, 0:1], in_=vcols[0])
        nc.vector.tensor_copy(out=vc2[:, 1:2], in_=vcols[1])
        w1t = wp.tile([128, 2, 128], BF16, tag="w1t", name="w1t")
        nc.vector.tensor_tensor(
            out=w1t[:], in0=sq[:].rearrange("p (g i) -> p g i", g=2),
            in1=vc2[:].unsqueeze(2).broadcast_to((128, 2, 128)), op=OP.mult)
        onesF = sb("onesF", [128, 1])
        nc.vector.memset(onesF[:], 1.0)
        lnt = wp.tile([128, 256], F32, tag="lnt", name="lnt")
        nc.scalar.activation(lnt[:], w1t[:], AF.Ln, bias=onesF[:, 0:1])
        rst = wp.tile([128, 256], BF16, tag="rst", name="rst")
        nc.scalar.activation(rst[:], lnt[:], AF.Exp, scale=-0.5)
        sT = wp.tile([128, 256], BF16, tag="sT", name="sT")
        nc.vector.tensor_tensor(out=sT[:], in0=p_znT[:], in1=rst[:], op=OP.mult)


        # ---------------- mask: c_max + mask columns (no gpsimd) -----------
        mred = sb("mred", [128, 1], wp)
        nc.vector.tensor_reduce(out=mred[:], in_=apk, op=OP.max, axis=AX.X,
                                apply_absolute_value=True)
        mrb = sb("mrb", [128, 1], dt=BF16)
        nc.vector.tensor_copy(out=mrb[:], in_=mred[:])
        p_mr = pt("p_mr", [1, 128], dt=BF16, pool=ps_t, tag="t")
        nc.tensor.transpose(p_mr[:], mrb[:], idB)
        cm1 = sb("cm1", [1, 1], wp)
        nc.vector.tensor_reduce(out=cm1[:], in_=p_mr[:], op=OP.max, axis=AX.X)
        rec1 = sb("rec1", [1, 1], wp)
        nc.vector.reciprocal(rec1[:], cm1[:])
        ge1 = sb("ge1", [1, 1], wp)
        nc.vector.tensor_scalar(out=ge1[:], in0=cm1[:], scalar1=1e-6,
                                scalar2=None, op0=OP.is_gt)
        f1c = sb("f1c", [1, 1], wp)
        nc.vector.tensor_scalar_add(out=f1c[:], in0=rec1[:], scalar1=-1.0)
        nc.vector.tensor_tensor(out=f1c[:], in0=ge1[:], in1=f1c[:], op=OP.mult)
        nc.vector.tensor_scalar_add(out=f1c[:], in0=f1c[:], scalar1=1.0)
        gof1 = sb("gof1", [1, 1], wp)
        nc.vector.tensor_scalar(out=gof1[:], in0=ge1[:], scalar1=-1e-3,
                                scalar2=1e-3 + 1e-9, op0=OP.mult, op1=OP.add)
        # one rank-1 broadcast for [f, g, alpha]
        fga = sb("fga", [1, 3], dt=BF16)
        nc.vector.tensor_copy(out=fga[:, 0:1], in_=f1c[:])
        nc.vector.tensor_copy(out=fga[:, 1:2], in_=gof1[:])
        nc.vector.tensor_copy(out=fga[:, 2:3],
                              in_=vp[0:1, VCOL["alpha"]:VCOL["alpha"] + 1])
        p_fga = pt("p_fga", [128, 3])
        nc.tensor.matmul(p_fga[:], ones1[:], fga[:], start=True, stop=True)
        fgacol = sb("fgacol", [128, 3])
        nc.vector.tensor_copy(out=fgacol[:], in_=p_fga[:])
        fcol, gcol, alcol = fgacol[:, 0:1], fgacol[:, 1:2], fgacol[:, 2:3]
        aab = sb("aab", [128, 2], wp)
        nc.scalar.activation(aab[:], apk[:, 517:519], AF.Abs)
        mkc = sb("mkc", [128, 2])
        nc.scalar.activation(mkc[:], aab[:], AF.Ln, bias=gcol, scale=fcol)

        # ---------------- label-score consts + label V row ----------------
        p_cr = pt("p_cr", [1, H])
        nc.tensor.matmul(p_cr[:], dcolb[:], th[:], start=True, stop=True)
        ecrow = sb("ecrow", [1, H], dt=BF16)
        nc.scalar.activation(ecrow[:], p_cr[:], AF.Exp,
                             bias=vp[0:1, VCOL["clog"]:VCOL["clog"] + 1])
        p_vdr = pt("p_vdr", [1, E])
        nc.tensor.matmul(p_vdr[:], dcolb[:], W2s("wv"), start=True, stop=True)
        ulcrow = sb("ulcrow", [1, E], dt=BF16)
        nc.vector.tensor_tensor(
            out=ulcrow[:].rearrange("p (g i) -> p g i", g=H),
            in0=p_vdr[:].rearrange("p (g i) -> p g i", g=H),
            in1=ecrow[:].unsqueeze(2).broadcast_to((1, H, 32)), op=OP.mult)

        # ---------------- UV chunks (g1 already in ut) ----------------
        uvs = []
        for i in range(2):
            p_uv = pt(f"p_uv{i}", [128, 128])
            nc.tensor.matmul(p_uv[:], uts[i][:], W2s("wv"), start=True, stop=True)
            uv = sb(f"uv{i}", [128, E], dt=BF16)
            nc.scalar.copy(uv[:], p_uv[:])
            uvs.append(uv)

        # ================= main phase =================
        pz4 = ps_z.tile([128, H], F32, tag="pz", name="pz4")
        p_on = ps_o.tile([128, 128], F32, tag="on", name="p_on")
        # label-position rank-1 init of the accumulators
        nc.tensor.matmul(pz4[:], ones1[:], ecrow[:], start=True, stop=False,
                         skip_group_check=True)
        nc.tensor.matmul(p_on[:], ones1[:], ulcrow[:], start=True, stop=False,
                         skip_group_check=True)
        # batched score map: one TT for all (chunk, head) pairs
        scTbig = wp.tile([128, 2, H, 128], BF16, tag="scTbig", name="scTbig")
        nc.vector.tensor_tensor(
            out=scTbig[:],
            in0=sT[:].rearrange("p (g i) -> p g i", g=2).unsqueeze(2)
                .broadcast_to((128, 2, H, 128)),
            in1=ac8b[:].rearrange("p (g h) -> p g h", g=2).unsqueeze(3)
                .broadcast_to((128, 2, H, 128)),
            op=OP.mult)
        eTbig = wp.tile([128, 2, H, 128], BF16, tag="eTbig", name="eTbig")
        for i in range(2):
            nc.scalar.activation(eTbig[:, i], scTbig[:, i], AF.Exp,
                                 bias=mkc[:, i:i + 1])
        wpreTbig = wp.tile([128, 2, H, 128], BF16, tag="wpreTbig",
                           name="wpreTbig")
        nc.vector.tensor_tensor(
            out=wpreTbig[:], in0=eTbig[:],
            in1=sT[:].rearrange("p (g i) -> p g i", g=2).unsqueeze(2)
                .broadcast_to((128, 2, H, 128)),
            op=OP.mult)
        # pz4 accums first: pz4 finishes before the numerator stream, so the
        # reciprocal overlaps the p_on matmuls.
        for i in range(2):
            for h in range(H):
                nc.tensor.matmul(pz4[:, h:h + 1], eTbig[:, i, h, :], onescol[:],
                                 start=False, stop=(i == 1 and h == H - 1),
                                 skip_group_check=True)
        for i in range(2):
            for h in range(H):
                nc.tensor.matmul(p_on[:, 32 * h:32 * (h + 1)],
                                 wpreTbig[:, i, h, :],
                                 uvs[i][:, 32 * h:32 * (h + 1)], start=False,
                                 stop=(i == 1 and h == H - 1),
                                 skip_group_check=True)

        # vccol folded into bo via Wo;  rescol for the final combine
        p_vc = pt("p_vc", [128, 1])
        nc.tensor.matmul(p_vc[:], W2s("wv"), W2s("beta1b", 1), start=True,
                         stop=True)
        vccol = sb("vccol", [E, 1], dt=BF16)
        nc.vector.tensor_scalar_add(out=vccol[:], in0=p_vc[:], scalar1=V("bv"))
        p_wovc = pt("p_wovc", [128, 1])
        nc.tensor.matmul(p_wovc[:], W3s("wo"), vccol[:], start=True, stop=True)
        boful = sb("boful", [E, 1], wp)
        nc.vector.tensor_scalar_add(out=boful[:], in0=p_wovc[:], scalar1=V("bo"))
        rescol = sb("rescol", [E, 1], wp)
        nc.vector.tensor_tensor(out=rescol[:], in0=boful[:], in1=V("b2"), op=OP.add)
        nc.vector.tensor_scalar(out=rescol[:], in0=rescol[:], scalar1=alcol,
                                scalar2=xlastF[:, 0:1], op0=OP.mult, op1=OP.add)

        # FFN bias consts + alpha-scaled W2
        b1ps = []
        for i, bn in enumerate(("b1a", "b1b")):
            p_b1 = pt(f"p_b1{i}", [128, 1])
            nc.tensor.matmul(p_b1[:], W3s("w1", 256)[:, 128 * i:128 * (i + 1)],
                             W2s("beta2b", 1), start=True, stop=True)
            b1p = sb(f"b1p{i}", [128, 1])
            nc.vector.tensor_scalar_add(out=b1p[:], in0=p_b1[:], scalar1=V(bn))
            b1ps.append(b1p)
        w2p = sb("w2p", [E, 2 * E], dt=BF16)
        nc.vector.tensor_scalar_mul(out=w2p[:], in0=W3s("w2", 256), scalar1=alcol)

        # normalize in [n, e]: oa = p_on * (1/Z)[n, h(e)]
        rz4 = sb("rz4", [128, H], wp)
        nc.vector.reciprocal(rz4[:], pz4[:])
        oab = sb("oab", [NP, E], dt=BF16)
        nc.vector.tensor_tensor(
            out=oab[:].rearrange("p (g i) -> p g i", g=H),
            in0=p_on[:].rearrange("p (g i) -> p g i", g=H),
            in1=rz4[:].unsqueeze(2).broadcast_to((128, H, 32)), op=OP.mult)
        p_oaT = pt("p_oaT", [E, NP], dt=BF16, pool=ps_t, tag="t")
        nc.tensor.transpose(p_oaT[:], oab[:], idB)
        oaT = sb("oaT", [E, 128], dt=BF16)
        nc.vector.tensor_copy(out=oaT[:], in_=p_oaT[:])

        # Wo
        p_wo = pt("p_wo", [128, 128])
        nc.tensor.matmul(p_wo[:], W3s("wo"), oaT[:], start=True, stop=True)
        ooT = sb("ooT", [E, 128], dt=BF16)
        nc.vector.tensor_scalar_add(out=ooT[:], in0=p_wo[:], scalar1=boful[:, 0:1])
        resid2 = sb("resid2", [E, 128], wp)
        nc.vector.tensor_scalar(out=resid2[:], in0=p_wo[:], scalar1=alcol,
                                scalar2=rescol[:, 0:1], op0=OP.mult, op1=OP.add)

        # LN over emb (stats in [n, e] layout)
        p_tn = pt("p_tn", [128, 128], dt=BF16, pool=ps_t, tag="t")
        nc.tensor.transpose(p_tn[:], ooT[:], idB)
        stO = sb("stO", [128, 6], wp); nc.vector.bn_stats(stO[:], p_tn[:])
        agO = sb("agO", [128, 2], wp); nc.vector.bn_aggr(agO[:], stO[:])
        vO = sb("vO", [128, 1], wp)
        nc.vector.tensor_scalar_add(out=vO[:], in0=agO[:, 1:2], scalar1=EPS)
        nmO = sb("nmO", [128, 1], wp)
        nc.vector.tensor_scalar_mul(out=nmO[:], in0=agO[:, 0:1], scalar1=-1.0)
        rstdO = sb("rstdO", [128, 1], wp)
        I32 = mybir.dt.int32
        nc.vector.tensor_scalar(out=rstdO[:].bitcast(I32), in0=vO[:].bitcast(I32),
                                scalar1=1, scalar2=None, op0=OP.arith_shift_right)
        nc.vector.tensor_scalar(out=rstdO[:].bitcast(I32), in0=rstdO[:].bitcast(I32),
                                scalar1=-1, scalar2=0x5F3759DF, op0=OP.mult,
                                op1=OP.add)
        nt = sb("nt", [128, 1], wp)
        nc.vector.tensor_tensor(out=nt[:], in0=rstdO[:], in1=rstdO[:], op=OP.mult)
        nc.vector.tensor_tensor(out=nt[:], in0=nt[:], in1=vO[:], op=OP.mult)
        nc.vector.tensor_scalar(out=nt[:], in0=nt[:], scalar1=-0.5,
                                scalar2=1.5, op0=OP.mult, op1=OP.add)
        nc.vector.tensor_tensor(out=rstdO[:], in0=rstdO[:], in1=nt[:], op=OP.mult)
        hpre = sb("hpre", [128, 128], dt=BF16)
        nc.vector.tensor_scalar(out=hpre[:], in0=p_tn[:], scalar1=nmO[:, 0:1],
                                scalar2=rstdO[:, 0:1], op0=OP.add, op1=OP.mult)
        p_ht = pt("p_ht", [128, 128], dt=BF16, pool=ps_t, tag="t")
        nc.tensor.transpose(p_ht[:], hpre[:], idB)
        hT = sb("hT", [128, 128], dt=BF16)   # g2 folded here
        nc.vector.tensor_scalar_mul(out=hT[:], in0=p_ht[:], scalar1=V("g2"))

        # FFN (both halves share one PSUM bank)
        p_f1 = pt("p_f1", [128, 256])
        nc.tensor.matmul(p_f1[:, 0:128], W3s("w1", 256)[:, 0:128], hT[:],
                         start=True, stop=True, skip_group_check=True)
        nc.tensor.matmul(p_f1[:, 128:256], W3s("w1", 256)[:, 128:256], hT[:],
                         start=True, stop=True, skip_group_check=True)
        gt = wp.tile([128, 2, 128], BF16, tag="gt", name="gt")
        nc.scalar.activation(gt[:, 0, :], p_f1[:, 0:128], AF.Gelu,
                             bias=b1ps[0][:, 0:1])
        nc.scalar.activation(gt[:, 1, :], p_f1[:, 128:256], AF.Gelu,
                             bias=b1ps[1][:, 0:1])
        p_y = pt("p_y", [128, 128])
        nc.tensor.matmul(p_y[:], w2p[:, 0:128], gt[:, 0, :], start=True, stop=False)
        nc.tensor.matmul(p_y[:], w2p[:, 128:256], gt[:, 1, :], start=False,
                         stop=True)

        # final combine [e, n]; host transposes
        zfT = sb("zfT", [128, 128], wp)
        nc.vector.tensor_tensor(out=zfT[:], in0=p_y[:], in1=resid2[:], op=OP.add)
        nc.sync.dma_start(out_ap, zfT[:])


_CACHE = {}


def _restrict_act_tables():
    """Limit the act-table-load pass to two sets so every non-Gelu activation
    (abs/copy/exp/identity/ln/square) resolves to one table and Gelu to the
    other -- avoids ~1.3us table reloads from per-function set churn."""
    import concourse.hw_specs as hws
    import concourse.bacc as bacc_mod
    orig = hws.get_activation_tables

    def patched(arch):
        t = orig(arch)
        keep = {}
        n_good = 0
        for name, fns in t.items():
            fnames = {f.name for f in fns}
            good = ("Ln" in fnames and "Exp" in fnames) or "Gelu" in fnames
            keep[name] = fns if good else set()   # keep positions for set ids
            n_good += bool(good)
        assert n_good >= 2, f"unexpected act table sets: {list(t)}"
        return keep

    bacc_mod.get_activation_tables = patched


def _get_nc():
    if "nc" in _CACHE:
        return _CACHE["nc"]
    _restrict_act_tables()
    nc = bacc.Bacc("TRN2", target_bir_lowering=False, debug=False,
                   num_devices=NCORES)
    d = {}
    for name, shape, dt in (("pack1", (128, P1_W), BF16),
                            ("Z", (NP, FD), BF16),
                            ("pack2", (128, P2_W), BF16),
                            ("pack3", (128, P3_W), BF16)):
        d[name] = nc.dram_tensor(name, list(shape), dt, kind="ExternalInput").ap()
    out_ap = nc.dram_tensor("out", [E, NP], F32, kind="ExternalOutput").ap()
    with tile.TileContext(nc) as tc:
        _body(tc, d, out_ap)
    nc.compile()
    _CACHE["nc"] = nc
    return nc


def _in_maps(inputs):
    import ml_dtypes
    bf = ml_dtypes.bfloat16
    a = {k: np.ascontiguousarray(np.asarray(v, dtype=np.float32))
         for k, v in inputs.items()}

    pack1 = np.zeros((128, P1_W), bf)
    pack1[:, P1["fe0"]:P1["fe0"] + 128] = a["feat_emb"][0:128].astype(bf)
    pack1[:, P1["fe1"]:P1["fe1"] + 128] = a["feat_emb"][128:256].astype(bf)
    vpack = np.zeros((128, VPACK_W), np.float32)
    vpack[:, VCOL["labT"]] = a["label_token"].reshape(E)
    for nm in ("bq", "bv", "bo", "b2", "g1", "beta1", "g2"):
        vpack[:, VCOL[nm]] = a[nm]
    vpack[:, VCOL["b1a"]] = a["b1"][0:128]
    vpack[:, VCOL["b1b"]] = a["b1"][128:256]
    vpack[0, VCOL["alpha"]] = float(np.asarray(a["alpha_res"]).reshape(-1)[0])
    vpack[:, VCOL["ceps"]] = EPS
    vpack[:, VCOL["ceps2"]] = EPS * EPS
    vpack[:, VCOL["clog"]] = LOG1P9
    pack1[:, P1["vp"]:P1["vp"] + 2 * VPACK_W] = np.ascontiguousarray(vpack).view(bf)
    pack1[0, P1["vrow"]:P1["vrow"] + 2 * E] = np.ascontiguousarray(
        a["label_token"].reshape(1, E)).view(bf)[0]
    pack1[0, P1["g1rowB"]:P1["g1rowB"] + E] = a["g1"].astype(bf)
    pack1[0, P1["b1rowB"]:P1["b1rowB"] + E] = a["beta1"].astype(bf)
    pack1[:, P1["idB"]:P1["idB"] + 128] = np.eye(128, dtype=np.float32).astype(bf)
    pack1[:, P1["wkT"]:P1["wkT"] + 128] = a["Wk"].T.astype(bf)
    pack1[:, P1["wq"]:P1["wq"] + 128] = a["Wq"].astype(bf)
    pack1[:, P1["hm4"]:P1["hm4"] + 4] = np.repeat(
        np.eye(4, dtype=np.float32), 32, axis=0).astype(bf)

    A = a["A_no_diag"]
    apack = np.zeros((128, APACK_W), np.float32)
    apack[:, 0:SEQ] = A[0:128]
    apack[:, SEQ:2 * SEQ] = A[128:256]
    apack[:, 514] = A[256, 0:128]
    apack[:, 515] = A[256, 128:256]
    apack[0, 516] = A[256, 256]
    apack[:, 517] = A[0:128, 256]
    apack[:, 518] = A[128:256, 256]
    pack2 = np.zeros((128, P2_W), bf)
    pack2[:, P2["apk"]:P2["apk"] + APACK_W] = apack.astype(bf)
    pack2[:, P2["wv"]:P2["wv"] + 128] = a["Wv"].astype(bf)
    pack2[:, P2["beta1b"]] = a["beta1"].astype(bf)
    pack2[:, P2["beta2b"]] = a["beta2"].astype(bf)

    pack3 = np.zeros((128, P3_W), bf)
    pack3[:, P3["w1"]:P3["w1"] + 256] = a["W1"].astype(bf)
    pack3[:, P3["w2"]:P3["w2"] + 256] = np.concatenate(
        [a["W2"][0:128], a["W2"][128:256]], axis=1).astype(bf)
    pack3[:, P3["wo"]:P3["wo"] + 128] = a["Wo"].astype(bf)

    maps = []
    for c in range(NCORES):
        m = {"pack1": pack1, "pack2": pack2, "pack3": pack3,
             "Z": np.ascontiguousarray(a["Z"][c * NP:(c + 1) * NP]).astype(bf)}
        maps.append(m)
    return maps


def run(inputs, trace=False):
    nc = _get_nc()
    res = run_bass_kernel_spmd(nc, _in_maps(inputs), core_ids=list(range(NCORES)),
                               trace=trace)
    out = np.concatenate([res.results[c]["out"].T for c in range(NCORES)], axis=0)
    return np.ascontiguousarray(out.astype(np.float32)), res


def kernel(**inputs):
    out, _ = run(inputs, trace=False)
    return out
